# revision 4
# baseline (speedup 1.0000x reference)
"""LSEP loss kernel for Trainium2 (8 NeuronCores, SPMD data-parallel).

loss = log1p( sum_i [ (sum_{c: t=0} exp(x_ic)) * (sum_{c: t=1} exp(-x_ic)) ] )

Strategy (v2, single-exp):  shard batch across 8 cores (4096 rows each),
partition p holds samples [32p, 32p+32) = 64000 u16 contiguous.  Both inputs
are DMA'd as uint16 views of their natural dtypes (host passes .view(u16),
zero-copy).  Per chunk of `ncols` samples:

  b  = x ^ (t << 15)   in the u16 lane domain      (DVE STT, 4x mode)
       t's u16 view is [t,0,t,0,...]; reading it through an AP shifted one
       lane right aligns t_j onto the HIGH half of f32 x_j, so t<<15 lands
       on the f32 sign bit: b = (-1)^t * x exactly.
  e  = exp(b)  bf16                                 (ScalarE, one big ACT)
  tc = bf16(t)  (0/1 mask, packed)                  (GpSimd copy, stride-2 src)
  u_k  = sum_c e            = s_neg + s_pos         (DVE tensor_scalar accum)
  sp_k = sum_c tc*e         = s_pos                 (DVE STT accum)

Epilogue: prod = (u-sp)*sp per sample, reduce, DMA [128,1] partial per core;
final scalar sum + log1p on host.  No masking constant needed; exact masks.
"""

import numpy as np

BATCH = 32768
C = 1000
N_CORES = 8
ROWS = BATCH // N_CORES          # 4096 rows per core
P = 128                          # SBUF partitions
SPR = ROWS // P                  # 32 samples per partition
# small chunks at both ends: fast pipeline ramp-in AND a short tail
CHUNKS = [1, 1, 1] + [2] * 13 + [1, 1, 1]  # sum == 32
SHIFT_IMM_MODE = "int_imm"       # "default" | "int_imm"
# walrus birverifier: "ScalarTensorTensor bitvec op must have ImmVal as
# integer and match the type of src and dst" -> emit a uint16-typed imm.

_CACHE = {}


def _build_nc():
    import concourse.bacc as bacc
    import concourse.mybir as mybir
    from concourse.tile import TileContext

    f32 = mybir.dt.float32
    bf16 = mybir.dt.bfloat16
    u16 = mybir.dt.uint16
    Exp = mybir.ActivationFunctionType.Exp
    Alu = mybir.AluOpType

    assert sum(CHUNKS) == SPR
    wmax = max(CHUNKS) * C           # f32 elems per chunk
    wmax2 = 2 * wmax                 # u16 elems per chunk

    nc = bacc.Bacc()
    x = nc.declare_dram_parameter("input", [ROWS, 2 * C], u16, isOutput=False)
    t = nc.declare_dram_parameter("target", [ROWS, 2 * C], u16, isOutput=False)
    out = nc.declare_dram_parameter("partial", [P, 1], f32, isOutput=True)

    # partition p holds samples [p*32, (p+1)*32), 64000 contiguous u16
    xv = x.rearrange("(p s) c -> p (s c)", p=P)
    tv = t.rearrange("(p s) c -> p (s c)", p=P)

    with TileContext(nc) as tc:
        with (
            tc.tile_pool(name="io", bufs=4) as io,
            tc.tile_pool(name="acc", bufs=1) as accp,
        ):
            ut = accp.tile([P, SPR], f32)     # per-sample u = s_neg + s_pos
            st = accp.tile([P, SPR], f32)     # per-sample s_pos
            scr_u = accp.tile([P, C], bf16)   # discarded main outs of accum ops
            scr_s = accp.tile([P, C], bf16)
            off = 0
            for ncols in CHUNKS:
                w = ncols * C
                w2 = 2 * w
                xt = io.tile([P, wmax2], u16, tag="x")
                tt = io.tile([P, wmax2 + 1], u16, tag="t")
                bt = io.tile([P, wmax2], u16, tag="b")
                et = io.tile([P, wmax], bf16, tag="e")
                tcm = io.tile([P, wmax], bf16, tag="tc")
                nc.gpsimd.memset(tt[:, 0:1], 0)  # pad lane (avoid stale reads)
                nc.sync.dma_start(tt[:, 1 : 1 + w2], tv[:, off * 2 * C : off * 2 * C + w2])
                nc.sync.dma_start(xt[:, :w2], xv[:, off * 2 * C : off * 2 * C + w2])
                # mask 0/1 -> packed bf16 (int->float convert on copy)
                tsrc = tt[:, 1 : 1 + w2].rearrange("p (c two) -> p c two", two=2)
                nc.gpsimd.tensor_copy(tcm[:, :w], tsrc[:, :, 0])
                # b = x ^ (t<<15): lane 2j+1 (f32 hi half) gets sign flipped
                if SHIFT_IMM_MODE == "default":
                    nc.vector.scalar_tensor_tensor(
                        bt[:, :w2], tt[:, 0:w2], 15, xt[:, :w2],
                        op0=Alu.logical_shift_left, op1=Alu.bitwise_xor,
                    )
                else:  # raw int-typed immediate fallback
                    eng = nc.vector
                    eng.add_instruction(
                        mybir.InstTensorScalarPtr(
                            name=nc.get_next_instruction_name(),
                            is_scalar_tensor_tensor=True,
                            op0=Alu.logical_shift_left,
                            op1=Alu.bitwise_xor,
                            ins=[
                                eng.lower_ap(tt[:, 0:w2]),
                                mybir.ImmediateValue(dtype=u16, value=15),
                                eng.lower_ap(xt[:, :w2]),
                            ],
                            outs=[eng.lower_ap(bt[:, :w2])],
                        )
                    )
                # e = exp(b) in one chunk-wide ACT, bf16 out
                nc.scalar.activation(et[:, :w], bt[:, :w2].bitcast(f32), Exp)
                for j in range(ncols):
                    k = off + j
                    esl = et[:, j * C : (j + 1) * C]
                    # u_k = sum e
                    nc.vector.tensor_scalar(
                        scr_u[:], esl, 1.0, None, Alu.mult, Alu.add,
                        accum_out=ut[:, k : k + 1],
                    )
                    # sp_k = sum t*e
                    nc.vector.scalar_tensor_tensor(
                        scr_s[:], tcm[:, j * C : (j + 1) * C], 1.0, esl,
                        op0=Alu.mult, op1=Alu.mult,
                        accum_out=st[:, k : k + 1],
                    )
                off += ncols
            # epilogue: prod = (u - sp) * sp = s_neg*s_pos, reduce to [128,1]
            d = accp.tile([P, SPR], f32)
            pr = accp.tile([P, SPR], f32)
            tot = accp.tile([P, 1], f32)
            nc.vector.tensor_tensor(d[:], ut[:], st[:], Alu.subtract)
            nc.vector.tensor_tensor(pr[:], d[:], st[:], Alu.mult)
            nc.vector.reduce_sum(tot[:], pr[:], axis=mybir.AxisListType.X)
            # out-DMA on the ACT HWDGE ring: the sync ring's FIFO still
            # holds input-DMA completions at this point
            nc.scalar.dma_start(out[:], tot[:])
    nc.compile()
    return nc


def _get_nc():
    if "nc" not in _CACHE:
        _CACHE["nc"] = _build_nc()
    return _CACHE["nc"]


def kernel(input, target):
    from concourse.bass_utils import run_bass_kernel_spmd

    x = np.ascontiguousarray(np.asarray(input, dtype=np.float32))
    t = np.ascontiguousarray(np.asarray(target, dtype=np.int32))
    assert x.shape == (BATCH, C) and t.shape == (BATCH, C)
    x16 = x.view(np.uint16)   # [BATCH, 2C], odd cols = bf16-truncated halves
    t16 = t.view(np.uint16)   # [BATCH, 2C], even cols = t exactly

    nc = _get_nc()
    in_maps = [
        {
            "input": x16[i * ROWS : (i + 1) * ROWS],
            "target": t16[i * ROWS : (i + 1) * ROWS],
        }
        for i in range(N_CORES)
    ]
    res = run_bass_kernel_spmd(nc, in_maps, list(range(N_CORES)))
    total = 0.0
    for r in res.results:
        total += float(np.sum(r["partial"].astype(np.float64)))
    return np.asarray([np.log1p(total)], dtype=np.float32)


# revision 6
# speedup vs baseline: 2.0227x; 2.0227x over previous
"""LSEP loss kernel for Trainium2 (8 NeuronCores, SPMD data-parallel).

loss = log1p( sum_i [ (sum_{c: t=0} exp(x_ic)) * (sum_{c: t=1} exp(-x_ic)) ] )

Strategy (v3, single-exp, 1x-rate DVE):  shard batch across 8 cores
(4096 rows each); partition p holds samples [32p, 32p+32) contiguous.
Per chunk of `ncols` samples:

  b  = x ^ (t << 31)          (DVE STT in i32 domain; flips f32 sign bit
                               where t==1, so b = (-1)^t * x exactly)
  per sample j:
    e_j = exp(b_j)  bf16, accum_out -> u_j = s_neg + s_pos   (ScalarE)
    sp_j = sum (t_j * 1.0) * e_j                             (DVE STT accum)

Epilogue: prod = (u-sp)*sp = s_neg*s_pos per sample, reduce, DMA [128,1]
partial per core; final scalar sum + log1p on host.  Masking is exact
(multiply by t), no BIG constant needed.

HW notes (measured): DVE executes 1 elem/lane/cycle at ~0.96GHz for all
dtypes (the cost model's 2x/4x 16-bit modes do not engage), so operate on
the fewest lanes possible (i32/f32, never u16 views).  GpSimd runs STT at
~0.4 eff; only the first ramp chunk's b-pass goes there (off critical path).
Inputs DMA at ~420 GB/s on the sync HWDGE ring; 32.77 MB/core => ~78 us
floor, which DVE (~78 us) and ACT (~42 us) just fit under.
"""

import numpy as np

BATCH = 32768
C = 1000
N_CORES = 8
ROWS = BATCH // N_CORES          # 4096 rows per core
P = 128                          # SBUF partitions
SPR = ROWS // P                  # 32 samples per partition
# small chunks at both ends: fast pipeline ramp-in AND a short tail
CHUNKS = [1, 1, 1] + [2] * 13 + [1, 1, 1]  # sum == 32
# GpSimd cannot run TensorScalarPtr (V3 ISA engine check) — keep empty.
GPSIMD_B_CHUNKS = set()

_CACHE = {}


def _build_nc():
    import concourse.bacc as bacc
    import concourse.mybir as mybir
    from concourse.tile import TileContext

    f32 = mybir.dt.float32
    bf16 = mybir.dt.bfloat16
    i32 = mybir.dt.int32
    Exp = mybir.ActivationFunctionType.Exp
    Alu = mybir.AluOpType

    assert sum(CHUNKS) == SPR
    wmax = max(CHUNKS) * C

    nc = bacc.Bacc()
    x = nc.declare_dram_parameter("input", [ROWS, C], i32, isOutput=False)
    t = nc.declare_dram_parameter("target", [ROWS, C], i32, isOutput=False)
    out = nc.declare_dram_parameter("partial", [P, 1], f32, isOutput=True)

    xv = x.rearrange("(p s) c -> p (s c)", p=P)
    tv = t.rearrange("(p s) c -> p (s c)", p=P)

    def stt_shift_xor(eng, out_ap, t_ap, x_ap):
        # b = (t << 31) ^ x.  walrus birverifier requires bitvec-op
        # immediates to be integer-typed and match src/dst dtype.
        eng.add_instruction(
            mybir.InstTensorScalarPtr(
                name=nc.get_next_instruction_name(),
                is_scalar_tensor_tensor=True,
                op0=Alu.logical_shift_left,
                op1=Alu.bitwise_xor,
                ins=[
                    eng.lower_ap(t_ap),
                    mybir.ImmediateValue(dtype=i32, value=31),
                    eng.lower_ap(x_ap),
                ],
                outs=[eng.lower_ap(out_ap)],
            )
        )

    with TileContext(nc) as tc:
        with (
            tc.tile_pool(name="io", bufs=4) as io,
            tc.tile_pool(name="acc", bufs=1) as accp,
        ):
            ut = accp.tile([P, SPR], f32)     # per-sample u = s_neg + s_pos
            st = accp.tile([P, SPR], f32)     # per-sample s_pos
            scr_s = accp.tile([P, C], bf16)   # discarded main out of sp accums
            off = 0
            for ci, ncols in enumerate(CHUNKS):
                w = ncols * C
                xt = io.tile([P, wmax], i32, tag="x")
                tt = io.tile([P, wmax], i32, tag="t")
                bt = io.tile([P, wmax], i32, tag="b")
                et = io.tile([P, wmax], bf16, tag="e")
                nc.sync.dma_start(tt[:, :w], tv[:, off * C : off * C + w])
                nc.sync.dma_start(xt[:, :w], xv[:, off * C : off * C + w])
                eng = nc.gpsimd if ci in GPSIMD_B_CHUNKS else nc.vector
                stt_shift_xor(eng, bt[:, :w], tt[:, :w], xt[:, :w])
                for j in range(ncols):
                    k = off + j
                    bsl = bt[:, j * C : (j + 1) * C].bitcast(f32)
                    esl = et[:, j * C : (j + 1) * C]
                    # e = exp(b); u_k = sum e  (free row-reduce on ScalarE)
                    nc.scalar.activation(
                        esl, bsl, Exp, accum_out=ut[:, k : k + 1]
                    )
                    # sp_k = sum t*e
                    nc.vector.scalar_tensor_tensor(
                        scr_s[:], tt[:, j * C : (j + 1) * C], 1.0, esl,
                        op0=Alu.mult, op1=Alu.mult,
                        accum_out=st[:, k : k + 1],
                    )
                off += ncols
            # epilogue: prod = (u - sp) * sp = s_neg*s_pos, reduce to [128,1]
            d = accp.tile([P, SPR], f32)
            pr = accp.tile([P, SPR], f32)
            tot = accp.tile([P, 1], f32)
            nc.vector.tensor_tensor(d[:], ut[:], st[:], Alu.subtract)
            nc.vector.tensor_tensor(pr[:], d[:], st[:], Alu.mult)
            nc.vector.reduce_sum(tot[:], pr[:], axis=mybir.AxisListType.X)
            # out-DMA on the ACT HWDGE ring: the sync ring's FIFO still
            # holds input-DMA completions at this point
            nc.scalar.dma_start(out[:], tot[:])
    nc.compile()
    return nc


def _get_nc():
    if "nc" not in _CACHE:
        _CACHE["nc"] = _build_nc()
    return _CACHE["nc"]


def kernel(input, target):
    from concourse.bass_utils import run_bass_kernel_spmd

    x = np.ascontiguousarray(np.asarray(input, dtype=np.float32))
    t = np.ascontiguousarray(np.asarray(target, dtype=np.int32))
    assert x.shape == (BATCH, C) and t.shape == (BATCH, C)
    xi = x.view(np.int32)   # raw-bits view; kernel flips the sign bit via xor

    nc = _get_nc()
    in_maps = [
        {
            "input": xi[i * ROWS : (i + 1) * ROWS],
            "target": t[i * ROWS : (i + 1) * ROWS],
        }
        for i in range(N_CORES)
    ]
    res = run_bass_kernel_spmd(nc, in_maps, list(range(N_CORES)))
    total = 0.0
    for r in res.results:
        total += float(np.sum(r["partial"].astype(np.float64)))
    return np.asarray([np.log1p(total)], dtype=np.float32)


# revision 9
# speedup vs baseline: 2.1147x; 1.0454x over previous
"""LSEP loss kernel for Trainium2 (8 NeuronCores, SPMD data-parallel).

loss = log1p( sum_i [ (sum_{c: t=0} exp(x_ic)) * (sum_{c: t=1} exp(-x_ic)) ] )

Strategy (v4, hybrid a/b forms):  shard batch across 8 cores (4096 rows
each); partition p holds samples [32p, 32p+32) contiguous.  Chunks of 2
samples alternate between two computation forms to balance DVE and ScalarE:

a-form (ScalarE-heavy; exact baseline masking):
  a = x - 50*t                       (DVE STT, f32)
  s_neg_k = sum exp(a)               (ACT per sample, accum_out)
  s_pos_k = sum exp(-a - 50)         (ACT per sample, scale=-1 bias=-50)

b-form (DVE-heavy; exact sign-flip masking):
  b = x ^ (t << 31)  = (-1)^t * x    (DVE STT, i32 bit domain)
  u_k  = sum exp(b)  (= s_neg+s_pos) (ACT per sample, accum_out; e kept bf16)
  sp_k = sum (t*1.0) * e             (DVE STT per sample, accum_out)
  s_neg_k = u_k - sp_k               (epilogue, stride-2 columns)

Epilogue: prod = s_neg*s_pos per sample, reduce, DMA [128,1] partial per
core; host sums 1024x8 partials and applies log1p.

HW facts this design is built on (measured on trn2 via ntff traces):
 - DVE: 1 elem/lane/cycle @0.96GHz for ALL dtypes (no 16-bit speedup).
 - ACT: ~1097ns per [128,1000] exp + 215ns accumulator read.
 - GpSimd: cannot run TensorScalarPtr (ISA); TT/CAST run at ~0.4 eff (slow).
 - DMA: ~427 GB/s steady on the sync HWDGE ring; 32.77MB/core => ~77us floor.
Engine budgets: DVE ~62us, ACT ~68us, both under the DMA floor.
"""

import numpy as np

BATCH = 32768
C = 1000
N_CORES = 8
ROWS = BATCH // N_CORES          # 4096 rows per core
P = 128                          # SBUF partitions
SPR = ROWS // P                  # 32 samples per partition
# small chunks at both ends: fast pipeline ramp-in AND a short tail
CHUNKS = [1, 1, 1] + [2] * 13 + [1, 1, 1]  # sum == 32

_CACHE = {}


def _build_nc():
    import concourse.bacc as bacc
    import concourse.mybir as mybir
    from concourse.tile import TileContext

    f32 = mybir.dt.float32
    bf16 = mybir.dt.bfloat16
    i32 = mybir.dt.int32
    Exp = mybir.ActivationFunctionType.Exp
    Alu = mybir.AluOpType

    assert sum(CHUNKS) == SPR
    wmax = max(CHUNKS) * C

    nc = bacc.Bacc()
    x = nc.declare_dram_parameter("input", [ROWS, C], i32, isOutput=False)
    t = nc.declare_dram_parameter("target", [ROWS, C], i32, isOutput=False)
    out = nc.declare_dram_parameter("partial", [P, 1], f32, isOutput=True)

    xv = x.rearrange("(p s) c -> p (s c)", p=P)
    tv = t.rearrange("(p s) c -> p (s c)", p=P)

    def stt_shift_xor(out_ap, t_ap, x_ap):
        # b = (t << 31) ^ x.  walrus birverifier requires bitvec-op
        # immediates to be integer-typed and match src/dst dtype.
        eng = nc.vector
        eng.add_instruction(
            mybir.InstTensorScalarPtr(
                name=nc.get_next_instruction_name(),
                is_scalar_tensor_tensor=True,
                op0=Alu.logical_shift_left,
                op1=Alu.bitwise_xor,
                ins=[
                    eng.lower_ap(t_ap),
                    mybir.ImmediateValue(dtype=i32, value=31),
                    eng.lower_ap(x_ap),
                ],
                outs=[eng.lower_ap(out_ap)],
            )
        )

    # alternate chunk forms, keeping the a/b sample counts balanced
    forms = []
    na = nb = 0
    for ncols in CHUNKS:
        if na <= nb:
            forms.append("a")
            na += ncols
        else:
            forms.append("b")
            nb += ncols

    with TileContext(nc) as tc:
        with (
            tc.tile_pool(name="io", bufs=6) as io,
            tc.tile_pool(name="acc", bufs=1) as accp,
        ):
            sn = accp.tile([P, SPR], f32)     # s_neg (a-form) / u (b-form)
            st = accp.tile([P, SPR], f32)     # s_pos
            scr_a = accp.tile([P, C], bf16)   # discarded ACT#2 main out
            scr_s = accp.tile([P, C], bf16)   # discarded sp-accum main out
            bneg = accp.tile([P, 1], f32)     # bias AP holding -50.0
            nc.vector.memset(bneg[:], -50.0)
            off = 0
            for ci, ncols in enumerate(CHUNKS):
                w = ncols * C
                form = forms[ci]
                xt = io.tile([P, wmax], i32, tag="x")
                tt = io.tile([P, wmax], i32, tag="t")
                bt = io.tile([P, wmax], i32, tag="b")
                nc.sync.dma_start(tt[:, :w], tv[:, off * C : off * C + w])
                nc.sync.dma_start(xt[:, :w], xv[:, off * C : off * C + w])
                if form == "a":
                    # a = t*(-50) + x, chunk-wide in f32
                    nc.vector.scalar_tensor_tensor(
                        bt[:, :w].bitcast(f32), tt[:, :w], -50.0,
                        xt[:, :w].bitcast(f32), op0=Alu.mult, op1=Alu.add,
                    )
                    for j in range(ncols):
                        k = off + j
                        asl = bt[:, j * C : (j + 1) * C].bitcast(f32)
                        nc.scalar.activation(
                            scr_a[:], asl, Exp, accum_out=sn[:, k : k + 1]
                        )
                        nc.scalar.activation(
                            scr_a[:], asl, Exp, scale=-1.0, bias=bneg[:],
                            accum_out=st[:, k : k + 1],
                        )
                else:
                    et = io.tile([P, wmax], bf16, tag="e")
                    stt_shift_xor(bt[:, :w], tt[:, :w], xt[:, :w])
                    for j in range(ncols):
                        k = off + j
                        bsl = bt[:, j * C : (j + 1) * C].bitcast(f32)
                        esl = et[:, j * C : (j + 1) * C]
                        nc.scalar.activation(
                            esl, bsl, Exp, accum_out=sn[:, k : k + 1]
                        )
                        nc.vector.scalar_tensor_tensor(
                            scr_s[:], tt[:, j * C : (j + 1) * C], 1.0, esl,
                            op0=Alu.mult, op1=Alu.mult,
                            accum_out=st[:, k : k + 1],
                        )
                off += ncols
            # epilogue: b-form columns hold u in sn -> s_neg = u - s_pos.
            # Correct ONLY those columns, then prod = sn*st, reduce, DMA out.
            d = accp.tile([P, SPR], f32)
            pr = accp.tile([P, SPR], f32)
            tot = accp.tile([P, 1], f32)
            bcols = []
            off = 0
            for ci, ncols in enumerate(CHUNKS):
                if forms[ci] == "b":
                    bcols.extend(range(off, off + ncols))
                off += ncols
            # contiguous runs of b-columns -> few strided sub-ops
            runs = []
            for k in bcols:
                if runs and runs[-1][1] == k:
                    runs[-1][1] = k + 1
                else:
                    runs.append([k, k + 1])
            for k0, k1 in runs:
                nc.vector.tensor_tensor(
                    d[:, k0:k1], sn[:, k0:k1], st[:, k0:k1], Alu.subtract
                )
                nc.vector.tensor_copy(sn[:, k0:k1], d[:, k0:k1])
            nc.vector.tensor_tensor(pr[:], sn[:], st[:], Alu.mult)
            nc.vector.reduce_sum(tot[:], pr[:], axis=mybir.AxisListType.X)
            # out-DMA on the ACT HWDGE ring: the sync ring's FIFO still
            # holds input-DMA completions at this point
            nc.scalar.dma_start(out[:], tot[:])
    nc.compile()
    return nc


def _get_nc():
    if "nc" not in _CACHE:
        _CACHE["nc"] = _build_nc()
    return _CACHE["nc"]


def kernel(input, target):
    from concourse.bass_utils import run_bass_kernel_spmd

    x = np.ascontiguousarray(np.asarray(input, dtype=np.float32))
    t = np.ascontiguousarray(np.asarray(target, dtype=np.int32))
    assert x.shape == (BATCH, C) and t.shape == (BATCH, C)
    xi = x.view(np.int32)   # raw-bits view; b-form flips the sign bit via xor

    nc = _get_nc()
    in_maps = [
        {
            "input": xi[i * ROWS : (i + 1) * ROWS],
            "target": t[i * ROWS : (i + 1) * ROWS],
        }
        for i in range(N_CORES)
    ]
    res = run_bass_kernel_spmd(nc, in_maps, list(range(N_CORES)))
    total = 0.0
    for r in res.results:
        total += float(np.sum(r["partial"].astype(np.float64)))
    return np.asarray([np.log1p(total)], dtype=np.float32)
